# revision 1
# baseline (speedup 1.0000x reference)
"""CrossStreamAttention Trainium2 kernel (8-core SPMD, data-parallel over query rows).

Reference computation (fp32):
    q = x_q @ Wq.T + bq            [N, D]
    k = x_kv @ Wk.T + bk           [M, D]
    v = x_kv @ Wv.T + bv           [M, D]
    out = softmax(q @ k.T / 16) @ v    [N, D]     (N = M = 8192, D = 256)

Sharding: x_q rows split 8 ways (1024 rows/core); x_kv + weights replicated;
each core computes its full 1024 x 8192 attention slab locally. No collectives.

Per-core device program:
  - load weights, transpose to [d_in, d_out] layout via PE transposes
  - transpose x_q / x_kv tiles via PE so activations sit [d, tokens] in SBUF
  - projections produce qT [d, n], kT [d, m] (biases fused into the ACT
    PSUM->SBUF copy) and v [m, d] with an extra all-ones column, so the
    attention P @ V matmul also produces the softmax denominator for free
  - attention: for each 128-row K/V chunk: S^T tile = kT-chunk.T @ qT
    (PSUM), exp on the scalar engine (scale=1/16 folded in, no max
    subtraction needed -- logits are O(1)), P tile goes straight back to
    the PE as the stationary operand of the P @ V accumulation
  - epilogue: divide by the ones-column denominator, add bv, store
  - all matmuls run as float32r (fp32 bits in memory, full-rate PE mode;
    every producer of a matmul operand writes the f32r-rounded dtype)
"""

import sys

for _p in ("/opt/trn_rl_repo", "/root/.axon_site/_ro/trn_rl_repo"):
    if _p not in sys.path:
        sys.path.append(_p)

import numpy as np

import concourse.bass as bass
import concourse.mybir as mybir
import concourse.tile as tile
from concourse import bacc
from concourse.masks import make_identity

N, M, D = 8192, 8192, 256
NCORES = 8
NL = N // NCORES          # query rows per core
P = 128                   # partition dim
KD = D // P               # 2 d-tiles of 128
MT = M // P               # 64 kv chunks of 128
NH = 512                  # n-half processed per attention pass
GK = 512                  # kv rows projected per group
NGK = M // GK             # 16 kv groups
SCALE = 1.0 / 16.0        # 1/sqrt(D)

FP32 = mybir.dt.float32
F32R = mybir.dt.float32r
AF = mybir.ActivationFunctionType


def _build_nc(reps=1, battn=1):
    nc = bacc.Bacc("TRN2", target_bir_lowering=False, debug=False,
                   num_devices=NCORES)

    xq_d = nc.dram_tensor("x_q", [NL, D], FP32, kind="ExternalInput")
    xkv_d = nc.dram_tensor("x_kv", [M, D], FP32, kind="ExternalInput")
    w_d = {
        "q": nc.dram_tensor("Wq", [D, D], FP32, kind="ExternalInput"),
        "k": nc.dram_tensor("Wk", [D, D], FP32, kind="ExternalInput"),
        "v": nc.dram_tensor("Wv", [D, D], FP32, kind="ExternalInput"),
    }
    bq_d = nc.dram_tensor("bq", [KD, P, 1], FP32, kind="ExternalInput")
    bk_d = nc.dram_tensor("bk", [KD, P, 1], FP32, kind="ExternalInput")
    bv_d = nc.dram_tensor("bv", [1, D], FP32, kind="ExternalInput")
    out_d = nc.dram_tensor("out", [NL, D], FP32, kind="ExternalOutput")

    with tile.TileContext(nc) as tc:
        _body(tc, xq_d, xkv_d, w_d, bq_d, bk_d, bv_d, out_d, reps=reps,
              battn=battn)
    nc.compile()
    return nc


def _body(tc, xq_d, xkv_d, w_d, bq_d, bk_d, bv_d, out_d, reps=1, battn=1):
    nc = tc.nc

    for rep in range(reps):
        _body_once(tc, xq_d, xkv_d, w_d, bq_d, bk_d, bv_d, out_d, rep, battn)


def _body_once(tc, xq_d, xkv_d, w_d, bq_d, bk_d, bv_d, out_d, rep, battn=1):
    nc = tc.nc

    with tc.tile_pool(name=f"const{rep}", bufs=1) as cpool:
        identity = cpool.tile([P, P], FP32, tag="identity", name="identity")
        make_identity(nc, identity[:])

        ones1 = cpool.tile([1, P], FP32, tag="ones1", name="ones1")
        nc.gpsimd.memset(ones1[:], 1.0)

        bq_sb = [cpool.tile([P, 1], FP32, tag=f"bq{a}", name=f"bq{a}") for a in range(KD)]
        bk_sb = [cpool.tile([P, 1], FP32, tag=f"bk{a}", name=f"bk{a}") for a in range(KD)]
        for a in range(KD):
            nc.sync.dma_start(bq_sb[a][:], bq_d[a])
            nc.sync.dma_start(bk_sb[a][:], bk_d[a])
        bv_row = cpool.tile([1, D], FP32, tag="bv_row", name="bv_row")
        nc.sync.dma_start(bv_row[:], bv_d[:])
        bv_bc = cpool.tile([P, D], FP32, tag="bv_bc", name="bv_bc")

        # weight tiles in [d_in, d_out] layout: wT[name][a] is [128, 256]
        wT = {nm: [cpool.tile([P, D], F32R, tag=f"wT{nm}{a}", name=f"wT{nm}{a}") for a in range(KD)]
              for nm in ("q", "k", "v")}

        qT = [cpool.tile([P, NL], F32R, tag=f"qT{a}", name=f"qT{a}") for a in range(KD)]
        kT = [cpool.tile([P, M], F32R, tag=f"kT{a}", name=f"kT{a}") for a in range(KD)]
        v_sb = cpool.tile([P, MT, D + 2], F32R, tag="v_sb", name="v_sb")
        # ones column -> P@V also accumulates the softmax denominator
        # (one pad column keeps the fp32r matmul free-dim even)
        nc.gpsimd.memset(v_sb[:, :, D:D + 1].bitcast(FP32), 1.0)
        nc.gpsimd.memset(v_sb[:, :, D + 1:D + 2].bitcast(FP32), 0.0)

        # ---------------- phase A: projections ----------------
        with tc.tile_pool(name="wload", bufs=2) as wload, \
             tc.tile_pool(name="xload", bufs=3) as xload, \
             tc.tile_pool(name="txT", bufs=2) as txT, \
             tc.tile_pool(name="tps", bufs=3, space="PSUM") as tps, \
             tc.tile_pool(name="pjps", bufs=2, space="PSUM") as pjps, \
             tc.tile_pool(name="vps", bufs=2, space="PSUM") as vps:

            # bv broadcast to all 128 partitions via a K=1 matmul
            bvp = pjps.tile([P, D], FP32, tag="pj", name="pj")
            nc.tensor.matmul(bvp[:], ones1[:], bv_row[:])
            nc.vector.tensor_copy(bv_bc[:], bvp[:])

            # weights: load rows [d_out, d_in], PE-transpose 128x128 blocks
            for nm in ("q", "k", "v"):
                for b in range(KD):
                    wrow = wload.tile([P, D], FP32, tag="wrow", name="wrow")
                    nc.sync.dma_start(wrow[:], w_d[nm][b * P:(b + 1) * P, :])
                    for a in range(KD):
                        ps = tps.tile([P, GK], FP32, tag="tps", name="tps")
                        nc.tensor.transpose(ps[:, 0:P],
                                            wrow[:, a * P:(a + 1) * P],
                                            identity[:])
                        nc.vector.tensor_copy(wT[nm][a][:, b * P:(b + 1) * P],
                                              ps[:, 0:P])

            _transpose_project(
                tc, nc, xq_d, NL, xload, txT, tps, pjps, None,
                wT["q"], bq_sb, qT, None, None, identity)

            _transpose_project(
                tc, nc, xkv_d, M, xload, txT, tps, pjps, vps,
                wT["k"], bk_sb, kT, wT["v"], v_sb, identity)

        # ---------------- phase B: attention ----------------
        with tc.tile_pool(name="ops", bufs=1, space="PSUM") as ops, \
             tc.tile_pool(name="stps", bufs=3, space="PSUM") as stps, \
             tc.tile_pool(name="ppool", bufs=4) as ppool, \
             tc.tile_pool(name="fin", bufs=2) as fin:

            for h in [hh % (NL // NH) for hh in range(battn * (NL // NH))]:
                o_ps = [ops.tile([P, D + 2], FP32, tag=f"o{j}", name=f"o{j}")
                        for j in range(NH // P)]
                for mi in range(MT):               # 64 kv chunks
                    st = stps.tile([P, NH], FP32, tag="st", name="st")
                    for a in range(KD):
                        nc.tensor.matmul(
                            st[:],
                            kT[a][:, mi * P:(mi + 1) * P],
                            qT[a][:, h * NH:(h + 1) * NH],
                            start=(a == 0), stop=(a == KD - 1))
                    p_t = ppool.tile([P, NH], F32R, tag="p", name="p")
                    nc.scalar.activation(p_t[:], st[:], AF.Exp, scale=SCALE)
                    for j in range(NH // P):
                        nc.tensor.matmul(
                            o_ps[j][:],
                            p_t[:, j * P:(j + 1) * P],
                            v_sb[:, mi, :],
                            start=(mi == 0), stop=(mi == MT - 1),
                            skip_group_check=True)

                for j in range(NH // P):
                    rec = fin.tile([P, 1], FP32, tag="rec", name="rec")
                    nc.vector.reciprocal(rec[:], o_ps[j][:, D:D + 1])
                    ob = fin.tile([P, D], FP32, tag="ob", name="ob")
                    nc.vector.tensor_scalar(ob[:], o_ps[j][:, 0:D], rec[:],
                                            None, op0=mybir.AluOpType.mult)
                    ob2 = fin.tile([P, D], FP32, tag="ob2", name="ob2")
                    nc.vector.tensor_add(ob2[:], ob[:], bv_bc[:])
                    r0 = h * NH + j * P
                    nc.sync.dma_start(out_d[r0:r0 + P, :], ob2[:])



def _transpose_project(tc, nc, x_d, nrows, xload, txT, tps, pjps, vps,
                       wT_main, bias_sb, outT, wT_v, v_sb, identity):
    """Stream x rows in 512-row groups: PE-transpose to [d, rows], then
    project.  outT[a][:, rows] = wT_main-rows.T @ xT (+bias, ACT copy).
    If wT_v/v_sb given, also emit v rows [m, d] chunks (DVE copy)."""
    ngroups = nrows // GK
    for g in range(ngroups):
        xT = txT.tile([P, KD, GK], F32R, tag="txT", name="txT")
        psa = [tps.tile([P, GK], FP32, tag="tps", name="tps") for _ in range(KD)]
        xg = xload.tile([P, GK // P, D], FP32, tag="xload", name="xload")
        nc.sync.dma_start(
            xg[:],
            x_d[g * GK:(g + 1) * GK, :].rearrange("(t p) d -> p t d", p=P))
        for t in range(GK // P):
            for a in range(KD):
                nc.tensor.transpose(psa[a][:, t * P:(t + 1) * P],
                                    xg[:, t, a * P:(a + 1) * P],
                                    identity[:])
        for a in range(KD):
            nc.vector.tensor_copy(xT[:, a, :], psa[a][:])

        for b in range(KD):
            pj = pjps.tile([P, GK], FP32, tag="pj", name="pj")
            for a in range(KD):
                nc.tensor.matmul(pj[:],
                                 wT_main[a][:, b * P:(b + 1) * P],
                                 xT[:, a, :],
                                 start=(a == 0), stop=(a == KD - 1))
            nc.scalar.activation(outT[b][:, g * GK:(g + 1) * GK], pj[:],
                                 AF.Identity, bias=bias_sb[b][:], scale=1.0)

        if wT_v is not None:
            for t in range(GK // P):
                vp = vps.tile([P, D], FP32, tag="vps", name="vps")
                for a in range(KD):
                    nc.tensor.matmul(vp[:],
                                     xT[:, a, t * P:(t + 1) * P],
                                     wT_v[a][:],
                                     start=(a == 0), stop=(a == KD - 1))
                mchunk = g * 4 + t
                if t % 2 == 0:
                    nc.vector.tensor_copy(v_sb[:, mchunk, 0:D], vp[:])
                else:
                    nc.scalar.copy(v_sb[:, mchunk, 0:D], vp[:])


# ---------------------------------------------------------------------------
# host-side: build once, run via a persistent sharded jit
# ---------------------------------------------------------------------------

_CACHE = {}


def _get_runner(reps=1, battn=1):
    key = f"runner{reps}_{battn}"
    if key in _CACHE:
        return _CACHE[key]

    import jax
    from jax.experimental.shard_map import shard_map
    from jax.sharding import Mesh, PartitionSpec

    from concourse import bass2jax
    from concourse.bass2jax import _bass_exec_p, install_neuronx_cc_hook

    install_neuronx_cc_hook()
    nc = _build_nc(reps=reps, battn=battn)

    partition_name = (nc.partition_id_tensor.name
                      if nc.partition_id_tensor else None)
    in_names, out_names, out_avals, zero_outs = [], [], [], []
    for alloc in nc.m.functions[0].allocations:
        if not isinstance(alloc, mybir.MemoryLocationSet):
            continue
        name = alloc.memorylocations[0].name
        if alloc.kind == "ExternalInput":
            if name != partition_name:
                in_names.append(name)
        elif alloc.kind == "ExternalOutput":
            shape = tuple(alloc.tensor_shape)
            dtype = mybir.dt.np(alloc.dtype)
            out_names.append(name)
            out_avals.append(jax.core.ShapedArray(shape, dtype))
            zero_outs.append(np.zeros(shape, dtype))
    n_params = len(in_names)
    all_in_names = list(in_names) + list(out_names)
    if partition_name is not None:
        all_in_names.append(partition_name)

    def _bodyfn(*args):
        operands = list(args)
        if partition_name is not None:
            operands.append(bass2jax.partition_id_tensor())
        outs = _bass_exec_p.bind(
            *operands,
            out_avals=tuple(out_avals),
            in_names=tuple(all_in_names),
            out_names=tuple(out_names),
            lowering_input_output_aliases=(),
            sim_require_finite=True,
            sim_require_nnan=True,
            nc=nc,
        )
        return tuple(outs)

    devices = jax.devices()[:NCORES]
    mesh = Mesh(np.asarray(devices), ("core",))
    n_outs = len(out_names)
    sharded = jax.jit(
        shard_map(_bodyfn, mesh=mesh,
                  in_specs=(PartitionSpec("core"),) * (n_params + n_outs),
                  out_specs=(PartitionSpec("core"),) * n_outs,
                  check_rep=False),
        keep_unused=True)

    runner = {
        "fn": sharded,
        "in_names": in_names,
        "out_names": out_names,
        "out_avals": out_avals,
        "zero_outs": zero_outs,
        "mesh": mesh,
    }
    _CACHE[key] = runner
    return runner


def _make_concat_inputs(x_q, x_kv, Wq, bq, Wk, bk, Wv, bv):
    """Per-core input dict -> concatenated global arrays (axis 0)."""
    f32 = np.float32
    per_core_shared = {
        "x_kv": np.ascontiguousarray(x_kv, dtype=f32),
        "Wq": np.ascontiguousarray(Wq, dtype=f32),
        "Wk": np.ascontiguousarray(Wk, dtype=f32),
        "Wv": np.ascontiguousarray(Wv, dtype=f32),
        "bq": np.ascontiguousarray(np.asarray(bq, dtype=f32).reshape(KD, P, 1)),
        "bk": np.ascontiguousarray(np.asarray(bk, dtype=f32).reshape(KD, P, 1)),
        "bv": np.ascontiguousarray(np.asarray(bv, dtype=f32).reshape(1, D)),
    }
    x_q = np.ascontiguousarray(x_q, dtype=f32)

    def core_input(name, c):
        if name == "x_q":
            return x_q[c * NL:(c + 1) * NL]
        return per_core_shared[name]

    runner = _get_runner()
    concat = []
    for name in runner["in_names"]:
        concat.append(np.concatenate(
            [core_input(name, c) for c in range(NCORES)], axis=0))
    return concat


def kernel(x_q, x_kv, Wq, bq, Wk, bk, Wv, bv):
    runner = _get_runner()
    concat_in = _make_concat_inputs(x_q, x_kv, Wq, bq, Wk, bk, Wv, bv)
    concat_zeros = [np.zeros((NCORES * z.shape[0], *z.shape[1:]), z.dtype)
                    for z in runner["zero_outs"]]
    outs = runner["fn"](*concat_in, *concat_zeros)
    idx = runner["out_names"].index("out")
    full = np.asarray(outs[idx])          # [8 * NL, D] row-concatenated
    return full.astype(np.float32)

